# revision 1
# baseline (speedup 1.0000x reference)
"""Bass/Trainium2 kernel for nn_GCNBase_5111011083135.

3-layer GCN: GCNConv->BN->ReLU, GCNConv->BN->ReLU, GCNConv->log_softmax.
N=100000 nodes, E=1600000 edges, D_in=D_h=128, D_out=40, 8 NeuronCores.

Strategy (graph/data parallel, nodes sharded over 8 cores):
- Host: fold all GCN normalization into per-edge weights
  ew'' = ew * dinv[dst] * dinv[src]  (self-loop edge: dinv[d]^2), where
  dinv = rsqrt(weighted_degree + 1). Nodes are relabeled (degree-balanced
  snake over tiles of 128) and padded to 102400 = 8 cores x 100 tiles x 128.
- Per layer l: table_l = h @ W_l per-shard ([f,n]-major matmul tiles) ->
  AllGather fp16 table [102400,128] -> per 2-tile psum-group: dma_gather
  source rows (int16 indices into 4 row-segments of the table), one
  selector matmul per 128-edge chunk (selector built by a single dual-op
  tensor_scalar is_equal x ew on DVE) accumulating agg^T [128f, 256n] in
  PSUM. BN stats via free-dim reduces + tiny AllReduce; BN+ReLU fused in
  one scalar-engine op. Final layer: z = agg2^T.T @ W2 + b2, log_softmax
  on chip. Output [12800, 40] fp32 per core, host unpermutes.
"""
import sys

if "/opt/trn_rl_repo" not in sys.path:
    sys.path.insert(0, "/opt/trn_rl_repo")

import numpy as np

P = 128
EPS = 1e-5


class Cfg:
    def __init__(self, ncores=8, tiles_per_core=100, nseg=4, n_real=100000,
                 d_in=128, d_out=40, gather_group=4, psum_group=2):
        self.ncores = ncores
        self.tiles_per_core = tiles_per_core
        self.nseg = nseg
        self.n_real = n_real
        self.d_in = d_in
        self.d_out = d_out
        self.gg = gather_group            # tiles per dma_gather group
        self.pg = psum_group              # tiles per psum accumulation group
        self.ntiles = ncores * tiles_per_core
        self.npad = self.ntiles * P
        assert self.npad % nseg == 0
        self.seg_rows = self.npad // nseg
        assert self.seg_rows <= 32768, "int16 index limit"
        assert tiles_per_core % gather_group == 0
        assert gather_group % psum_group == 0
        self.ngroups = tiles_per_core // gather_group      # per core
        self.npg = tiles_per_core // psum_group            # psum groups/core
        self.pg_nodes = psum_group * P                     # 256


def preprocess(cfg, x, edge_index, edge_weight):
    """Host-side graph prep. Returns per-core input dicts + unpermute info."""
    N = cfg.n_real
    src = np.asarray(edge_index[0], dtype=np.int64)
    dst = np.asarray(edge_index[1], dtype=np.int64)
    ew = np.asarray(edge_weight, dtype=np.float64)

    degw = np.bincount(dst, weights=ew, minlength=N) + 1.0
    dinv = 1.0 / np.sqrt(degw)                              # [N]

    # --- balanced relabeling: sort by in-degree desc, snake over tiles ---
    cnt = np.bincount(dst, minlength=N) + 1
    order = np.argsort(-cnt, kind="stable")                  # node ids by load desc
    r = np.arange(N)
    posr = r % cfg.ntiles
    rnd = r // cfg.ntiles
    tile_of_rank = np.where(rnd % 2 == 0, posr, cfg.ntiles - 1 - posr)
    newpos = np.empty(N, dtype=np.int64)
    newpos[order] = tile_of_rank * P + rnd                   # new node id

    # --- edge list incl. self-loops, with folded weights ---
    s_n = newpos[src]
    d_n = newpos[dst]
    eww = ew * dinv[dst] * dinv[src]
    self_s = newpos[np.arange(N)]
    self_w = dinv * dinv
    s_all = np.concatenate([s_n, self_s])
    d_all = np.concatenate([d_n, self_s])
    w_all = np.concatenate([eww, self_w]).astype(np.float32)

    # --- bucket edges by (psum-group, segment) ---
    pg_of = d_all >> int(np.log2(cfg.pg_nodes))              # global psum-group id
    seg_of = s_all // cfg.seg_rows
    npg_tot = cfg.npad // cfg.pg_nodes
    bucket = pg_of * cfg.nseg + seg_of                       # [(pg, s)]
    order_e = np.argsort(bucket, kind="stable")
    s_all, d_all, w_all, bucket = (a[order_e] for a in (s_all, d_all, w_all, bucket))
    counts = np.bincount(bucket, minlength=npg_tot * cfg.nseg).reshape(npg_tot, cfg.nseg)

    # uniform block counts across cores: b_star[local_pg][s]
    npg_core = cfg.npg
    counts_c = counts.reshape(cfg.ncores, npg_core, cfg.nseg)
    blocks_c = -(-counts_c // P)                             # ceil
    b_star = blocks_c.max(axis=0)                            # [npg_core, nseg]

    # --- slot layout: call order is (g, s, pg-within-g); blocks within ---
    # bucket (local pg, s) -> slot offset (in blocks) within the core stream
    pg_per_g = cfg.gg // cfg.pg
    off = np.zeros((npg_core, cfg.nseg), dtype=np.int64)
    call_len = np.zeros((cfg.ngroups, cfg.nseg), dtype=np.int64)  # blocks per call
    pos = 0
    for g in range(cfg.ngroups):
        for s in range(cfg.nseg):
            for j in range(pg_per_g):
                lpg = g * pg_per_g + j
                off[lpg, s] = pos
                pos += int(b_star[lpg, s])
            call_len[g, s] = sum(int(b_star[g * pg_per_g + j, s]) for j in range(pg_per_g))
    m_blocks = pos                                           # blocks per core

    # --- fill per-core aux arrays ---
    srcw = np.zeros((cfg.ncores, 16, m_blocks * 8), dtype=np.int16)
    dstrel = np.zeros((cfg.ncores, P, m_blocks), dtype=np.float32)
    ewa = np.zeros((cfg.ncores, P, m_blocks), dtype=np.float32)

    # per-edge slot index (vectorized): slot within bucket
    bc = np.bincount(bucket, minlength=npg_tot * cfg.nseg)
    bstart = np.concatenate([[0], np.cumsum(bc)])[:-1]
    within = np.arange(len(s_all)) - bstart[bucket]
    core_e = bucket // (npg_core * cfg.nseg)
    lpg_e = (bucket // cfg.nseg) % npg_core
    seg_e = bucket % cfg.nseg
    slot = off[lpg_e, seg_e] * P + within                    # slot within core stream
    p_e = (slot % P).astype(np.int64)
    blk_e = (slot // P).astype(np.int64)
    srcseg = (s_all % cfg.seg_rows).astype(np.int16)
    drel = (d_all % cfg.pg_nodes).astype(np.float32)

    for c in range(cfg.ncores):
        m = core_e == c
        dstrel[c, p_e[m], blk_e[m]] = drel[m]
        ewa[c, p_e[m], blk_e[m]] = w_all[m]
        # wrapped idx: logical i (= blk*128 + p) -> [i%16, i//16]
        i_log = blk_e[m] * P + p_e[m]
        srcw[c, i_log % 16, i_log // 16] = srcseg[m]
    srcw = np.broadcast_to(srcw[:, None, :, :], (cfg.ncores, 8, 16, m_blocks * 8)) \
        .reshape(cfg.ncores, P, m_blocks * 8).copy()

    # --- feature shards, transposed tiles ---
    nper = cfg.tiles_per_core * P
    x_pad = np.zeros((cfg.npad, cfg.d_in), dtype=np.float16)
    x_pad[newpos] = np.asarray(x, dtype=np.float16)
    xT = np.zeros((cfg.ncores, cfg.tiles_per_core, cfg.d_in, P), dtype=np.float16)
    for c in range(cfg.ncores):
        sh = x_pad[c * nper:(c + 1) * nper].reshape(cfg.tiles_per_core, P, cfg.d_in)
        xT[c] = sh.transpose(0, 2, 1)

    iota = np.broadcast_to(
        np.arange(cfg.pg_nodes, dtype=np.float16), (P, cfg.pg_nodes)).copy()

    meta = dict(b_star=b_star, call_len=call_len, off=off, m_blocks=m_blocks)
    per_core = []
    for c in range(cfg.ncores):
        per_core.append(dict(
            xT=xT[c].reshape(cfg.tiles_per_core * cfg.d_in, P),
            srcw=srcw[c], dstrel=dstrel[c], ewa=ewa[c], iota=iota,
        ))
    return per_core, newpos, meta


def build_program(cfg, meta, no_collectives=False, rep=1, nsteps=8,
                  ablate=()):
    """Build the SPMD Bass program. Structure depends on cfg + meta block
    counts (data-dependent but identical across cores). no_collectives
    replaces collectives with plain DMAs (for single-core TimelineSim).
    rep>1 repeats the whole pipeline for wall-clock timing."""
    import concourse.bass as bass
    import concourse.bacc as bacc
    import concourse.tile as tile
    import concourse.mybir as mybir
    from concourse.masks import make_identity

    f16, f32, i16 = mybir.dt.float16, mybir.dt.float32, mybir.dt.int16
    AT = mybir.ActivationFunctionType
    OP = mybir.AluOpType
    b_star = meta["b_star"]; call_len = meta["call_len"]
    off = meta["off"]; m_blocks = meta["m_blocks"]
    T = cfg.tiles_per_core
    D, DO = cfg.d_in, cfg.d_out
    PGN = cfg.pg_nodes
    pg_per_g = cfg.gg // cfg.pg
    inv_n = 1.0 / cfg.n_real
    RG = [list(range(cfg.ncores))]

    nc = bacc.Bacc("TRN2", target_bir_lowering=False, debug=False,
                   num_devices=1 if no_collectives else cfg.ncores)

    # ---- I/O ----
    xT_d = nc.dram_tensor("xT", [T * D, P], f16, kind="ExternalInput")
    srcw_d = nc.dram_tensor("srcw", [P, m_blocks * 8], i16, kind="ExternalInput")
    dstrel_d = nc.dram_tensor("dstrel", [P, m_blocks], f32, kind="ExternalInput")
    ewa_d = nc.dram_tensor("ewa", [P, m_blocks], f32, kind="ExternalInput")
    iota_d = nc.dram_tensor("iota", [P, PGN], f16, kind="ExternalInput")
    W0_d = nc.dram_tensor("W0", [D, D], f32, kind="ExternalInput")
    W1_d = nc.dram_tensor("W1", [D, D], f32, kind="ExternalInput")
    W2_d = nc.dram_tensor("W2", [D, DO], f32, kind="ExternalInput")
    b2_d = nc.dram_tensor("b2", [1, DO], f32, kind="ExternalInput")
    g0_d = nc.dram_tensor("g0", [P, 1], f32, kind="ExternalInput")
    be0_d = nc.dram_tensor("be0", [P, 1], f32, kind="ExternalInput")
    g1_d = nc.dram_tensor("g1", [P, 1], f32, kind="ExternalInput")
    be1_d = nc.dram_tensor("be1", [P, 1], f32, kind="ExternalInput")
    out_d = nc.dram_tensor("out", [T * P, DO], f32, kind="ExternalOutput")

    ybounce = nc.dram_tensor("ybounce", [T * P, D], f16, kind="Internal")
    yfull = nc.dram_tensor("yfull", [cfg.npad, D], f16, kind="Internal",
                           addr_space="Shared")
    stat_in = [nc.dram_tensor(f"stat_in{l}", [P, 2], f32, kind="Internal")
               for l in range(2)]
    stat_out = [nc.dram_tensor(f"stat_out{l}", [P, 2], f32, kind="Internal",
                               addr_space="Shared") for l in range(2)]

    with tile.TileContext(nc) as tc:
        with (
            tc.tile_pool(name="res", bufs=1) as res,       # resident
            tc.tile_pool(name="yg", bufs=2) as ygp,        # gather buffers
            tc.tile_pool(name="sel", bufs=4) as selp,      # selector tiles
            tc.tile_pool(name="work", bufs=3) as work,     # small transient
            tc.tile_pool(name="agps", bufs=2, space="PSUM") as agps,
            tc.tile_pool(name="yps", bufs=2, space="PSUM") as yps,
        ):
            # ---- resident loads ----
            xT = res.tile([P, T * D], f16)
            nc.sync.dma_start(
                xT[:].rearrange("f (t n) -> f t n", n=P),
                xT_d[:].rearrange("(t f) n -> f t n", f=D))
            # xT column block t = [128f, 128n]? NO: laid out f-major:
            # xT[p=f, t*P + n] -- see rearrange above.
            srcw = res.tile([P, m_blocks * 8], i16)
            nc.sync.dma_start(srcw[:], srcw_d[:])
            dstrel = res.tile([P, m_blocks], f32)
            nc.sync.dma_start(dstrel[:], dstrel_d[:])
            ewa = res.tile([P, m_blocks], f32)
            nc.sync.dma_start(ewa[:], ewa_d[:])
            iota = res.tile([P, PGN], f16)
            nc.sync.dma_start(iota[:], iota_d[:])
            Wl = []
            for Wd, dd in ((W0_d, D), (W1_d, D), (W2_d, DO)):
                w32 = work.tile([P, dd], f32, tag="w32")
                nc.sync.dma_start(w32[:], Wd[:])
                w16 = res.tile([P, dd], f16, name=f"W16_{len(Wl)}", tag=f"W{dd}_{len(Wl)}")
                nc.vector.tensor_copy(w16[:], w32[:])
                Wl.append(w16)
            b2b = res.tile([P, DO], f32)
            nc.sync.dma_start(b2b[:], b2_d[:].to_broadcast([P, DO]))
            bng = []
            for td in (g0_d, be0_d, g1_d, be1_d):
                t_ = res.tile([P, 1], f32, name=f"bnp{len(bng)}", tag=f"bn{len(bng)}")
                nc.sync.dma_start(t_[:], td[:])
                bng.append(t_)
            ident = res.tile([P, P], f16)
            make_identity(nc, ident[:])

            agg = res.tile([P, T * P], f16)               # agg^T, col-block t
            y_sb = res.tile([P, T * P], f16)              # y node-major col-block t
            out_sb = res.tile([P, T * DO], f32)
            stat_s = res.tile([P, cfg.npg], f32)
            stat_q = res.tile([P, cfg.npg], f32)
            sbn = [res.tile([P, 1], f32, name=f"sbn{l}", tag=f"sbn{l}") for l in range(2)]
            bbn = [res.tile([P, 1], f32, name=f"bbn{l}", tag=f"bbn{l}") for l in range(2)]

            # ---------------- helper phases ----------------
            def y_pass(layer):
                """table_l tiles into y_sb (node-major f16 col-blocks)."""
                for t in range(T):
                    if layer == 0:
                        lhsT = xT[:, t * D:(t + 1) * D]
                    else:
                        hT = work.tile([P, P], f16, tag="hT")
                        nc.scalar.activation(
                            out=hT[:], in_=agg[:, t * P:(t + 1) * P],
                            func=AT.Relu, bias=bbn[layer - 1][:],
                            scale=sbn[layer - 1][:])
                        lhsT = hT[:]
                    if layer < 2:
                        ps = yps.tile([P, D], f32, space="PSUM", tag="yps")
                        nc.tensor.matmul(out=ps[:], lhsT=lhsT, rhs=Wl[layer][:],
                                         start=True, stop=True)
                        nc.scalar.activation(
                            out=y_sb[:, t * P:(t + 1) * P], in_=ps[:], func=AT.Copy)
                    else:
                        # transpose h^T -> node-major h
                        ps = yps.tile([P, P], f16, space="PSUM", tag="ypsT")
                        nc.tensor.transpose(out=ps[:], in_=lhsT, identity=ident[:])
                        nc.scalar.activation(
                            out=y_sb[:, t * P:(t + 1) * P], in_=ps[:], func=AT.Copy)
                nc.gpsimd.dma_start(
                    ybounce[:].rearrange("(t n) f -> n t f", n=P),
                    y_sb[:].rearrange("n (t f) -> n t f", f=D))
                if no_collectives:
                    nc.gpsimd.dma_start(yfull[:T * P, :], ybounce[:])
                else:
                    nc.gpsimd.collective_compute(
                        "AllGather", mybir.AluOpType.bypass, replica_groups=RG,
                        ins=[ybounce[:]], outs=[yfull[:]])

            def agg_pass(layer):
                """gather + selector matmuls -> agg (and stats for layer<2)."""
                shared_S = None
                if "share_sel" in ablate:
                    shared_S = res.tile([P, PGN], f16, name="sharedS")
                    nc.vector.tensor_scalar(
                        out=shared_S[:], in0=iota[:], scalar1=dstrel[:, 0:1],
                        scalar2=ewa[:, 0:1], op0=OP.is_equal, op1=OP.mult)
                for g in range(cfg.ngroups):
                    gblocks = int(call_len[g].sum())
                    ygb = ygp.tile([P, gblocks * D], f16, tag="ygb")
                    g0off = off[g * pg_per_g, 0]           # first block of group
                    boff = 0
                    for s in range(cfg.nseg):
                        nb = int(call_len[g, s])
                        if nb == 0:
                            continue
                        nidx = nb * P
                        blk0 = off[g * pg_per_g, s] if pg_per_g else 0
                        # idx cols for this call: blocks [blk0, blk0+nb)
                        if "skip_gather" not in ablate:
                            nc.gpsimd.dma_gather(
                                ygb[:, boff * D:(boff + nb) * D]
                                    .rearrange("p (b f) -> p b f", f=D),
                                yfull[s * cfg.seg_rows:(s + 1) * cfg.seg_rows, :],
                                srcw[:, blk0 * 8:(blk0 + nb) * 8],
                                nidx, nidx, D, elem_step=D, single_packet=False,
                            )
                        elif boff == 0:
                            # minimal write so the tile gets allocated
                            nc.gpsimd.dma_gather(
                                ygb[:, 0:D].rearrange("p (b f) -> p b f", f=D),
                                yfull[0:cfg.seg_rows, :], srcw[:, 0:8],
                                P, P, D, elem_step=D, single_packet=False,
                            )
                        boff += nb
                    # consume: psum groups
                    for j in range(pg_per_g):
                        lpg = g * pg_per_g + j
                        ps = agps.tile([P, PGN], f32, space="PSUM", tag="agps")
                        first = True
                        nblk = int(b_star[lpg].sum())
                        done = 0
                        for s in range(cfg.nseg):
                            nb = int(b_star[lpg, s])
                            for b in range(nb):
                                blk = off[lpg, s] + b      # global block id
                                # position inside ygb: blocks laid in call order
                                ybpos = blk - g0off
                                if "share_sel" in ablate:
                                    S = shared_S
                                else:
                                    S = selp.tile([P, PGN], f16, tag="S")
                                    nc.vector.tensor_scalar(
                                        out=S[:], in0=iota[:],
                                        scalar1=dstrel[:, blk:blk + 1],
                                        scalar2=ewa[:, blk:blk + 1],
                                        op0=OP.is_equal, op1=OP.mult)
                                done += 1
                                if "one_mm" not in ablate or done == 1:
                                    nc.tensor.matmul(
                                        out=ps[:],
                                        lhsT=ygb[:, ybpos * D:(ybpos + 1) * D],
                                        rhs=S[:], start=first,
                                        stop=(done == nblk or "one_mm" in ablate),
                                    )
                                first = False
                        cslice = slice(lpg * PGN, (lpg + 1) * PGN)
                        if layer < 2:
                            nc.vector.tensor_copy(agg[:, cslice], ps[:])
                            nc.vector.tensor_reduce(
                                out=stat_s[:, lpg:lpg + 1], in_=agg[:, cslice],
                                axis=mybir.AxisListType.X, op=OP.add)
                            scr = work.tile([P, PGN], f16, tag="sqscr")
                            nc.scalar.activation(
                                out=scr[:], in_=agg[:, cslice], func=AT.Square,
                                accum_out=stat_q[:, lpg:lpg + 1])
                        else:
                            # final layer: z = aggT.T @ W2 + b2, log_softmax
                            t2 = work.tile([P, PGN], f16, tag="t2")
                            nc.vector.tensor_copy(t2[:], ps[:])
                            for k in range(cfg.pg):
                                t_ = lpg * cfg.pg + k
                                zp = yps.tile([P, DO], f32, space="PSUM", tag="zps")
                                nc.tensor.matmul(
                                    out=zp[:], lhsT=t2[:, k * P:(k + 1) * P],
                                    rhs=Wl[2][:], start=True, stop=True)
                                z = work.tile([P, DO], f32, tag="z")
                                nc.vector.tensor_tensor(
                                    out=z[:], in0=zp[:], in1=b2b[:], op=OP.add)
                                negm = work.tile([P, 1], f32, tag="negm")
                                nc.vector.tensor_reduce(
                                    out=negm[:], in_=z[:],
                                    axis=mybir.AxisListType.X, op=OP.max,
                                    negate=True)
                                scr = work.tile([P, DO], f32, tag="escr")
                                sume = work.tile([P, 1], f32, tag="sume")
                                nc.scalar.activation(
                                    out=scr[:], in_=z[:], func=AT.Exp,
                                    bias=negm[:], accum_out=sume[:])
                                lse = work.tile([P, 1], f32, tag="lse")
                                nc.scalar.activation(
                                    out=lse[:], in_=sume[:], func=AT.Ln)
                                nc.vector.tensor_scalar(
                                    out=out_sb[:, t_ * DO:(t_ + 1) * DO],
                                    in0=z[:], scalar1=negm[:], scalar2=lse[:],
                                    op0=OP.add, op1=OP.subtract)

            def bn_stats(layer):
                ssum = work.tile([P, 1], f32, tag="ssum")
                sq = work.tile([P, 1], f32, tag="sq")
                nc.vector.tensor_reduce(out=ssum[:], in_=stat_s[:],
                                        axis=mybir.AxisListType.X,
                                        op=mybir.AluOpType.add)
                nc.vector.tensor_reduce(out=sq[:], in_=stat_q[:],
                                        axis=mybir.AxisListType.X,
                                        op=mybir.AluOpType.add)
                pack = work.tile([P, 2], f32, tag="pack")
                nc.vector.tensor_copy(pack[:, 0:1], ssum[:])
                nc.vector.tensor_copy(pack[:, 1:2], sq[:])
                nc.gpsimd.dma_start(stat_in[layer][:], pack[:])
                if no_collectives:
                    nc.gpsimd.dma_start(stat_out[layer][:], stat_in[layer][:])
                else:
                    nc.gpsimd.collective_compute(
                        "AllReduce", mybir.AluOpType.add, replica_groups=RG,
                        ins=[stat_in[layer][:]], outs=[stat_out[layer][:]])
                red = work.tile([P, 2], f32, tag="red")
                nc.sync.dma_start(red[:], stat_out[layer][:])
                mean = work.tile([P, 1], f32, tag="mean")
                nc.vector.tensor_scalar_mul(mean[:], red[:, 0:1], inv_n)
                msq = work.tile([P, 1], f32, tag="msq")
                nc.vector.tensor_scalar_mul(msq[:], red[:, 1:2], inv_n)
                var = work.tile([P, 1], f32, tag="var")
                nc.vector.tensor_tensor(out=var[:], in0=mean[:], in1=mean[:],
                                        op=mybir.AluOpType.mult)
                nc.vector.tensor_tensor(out=var[:], in0=msq[:], in1=var[:],
                                        op=mybir.AluOpType.subtract)
                nc.vector.tensor_scalar_add(var[:], var[:], EPS)
                sd = work.tile([P, 1], f32, tag="sd")
                nc.scalar.sqrt(sd[:], var[:])
                rsd = work.tile([P, 1], f32, tag="rsd")
                nc.vector.reciprocal(rsd[:], sd[:])
                nc.vector.tensor_tensor(out=sbn[layer][:], in0=bng[2 * layer][:],
                                        in1=rsd[:], op=mybir.AluOpType.mult)
                mb = work.tile([P, 1], f32, tag="mb")
                nc.vector.tensor_tensor(out=mb[:], in0=mean[:], in1=sbn[layer][:],
                                        op=mybir.AluOpType.mult)
                nc.vector.tensor_tensor(out=bbn[layer][:],
                                        in0=bng[2 * layer + 1][:], in1=mb[:],
                                        op=mybir.AluOpType.subtract)

            # ---------------- the program ----------------
            if nsteps < 8:
                nc.vector.memset(out_sb[:], 0.0)
            steps = [
                lambda: y_pass(0), lambda: agg_pass(0), lambda: bn_stats(0),
                lambda: y_pass(1), lambda: agg_pass(1), lambda: bn_stats(1),
                lambda: y_pass(2), lambda: agg_pass(2),
            ]
            for _r in range(rep):
                for st in steps[:nsteps]:
                    st()
            nc.sync.dma_start(
                out_d[:].rearrange("(t n) c -> n t c", n=P),
                out_sb[:].rearrange("n (t c) -> n t c", c=DO))

    nc.compile()
    return nc


_CACHE = {}


def _run(cfg, inputs):
    from concourse.bass_utils import run_bass_kernel_spmd

    x = inputs["x"]
    per_core, newpos, meta = preprocess(
        cfg, x, inputs["edge_index"], inputs["edge_weight"])

    key = (cfg.ncores, cfg.tiles_per_core, meta["m_blocks"],
           tuple(meta["b_star"].ravel().tolist()))
    if key not in _CACHE:
        _CACHE[key] = build_program(cfg, meta)
    nc = _CACHE[key]

    shared = dict(
        W0=np.asarray(inputs["W0"], np.float32),
        W1=np.asarray(inputs["W1"], np.float32),
        W2=np.asarray(inputs["W2"], np.float32),
        b2=np.asarray(inputs["b2"], np.float32).reshape(1, -1),
        g0=np.asarray(inputs["g0"], np.float32).reshape(-1, 1),
        be0=np.asarray(inputs["be0"], np.float32).reshape(-1, 1),
        g1=np.asarray(inputs["g1"], np.float32).reshape(-1, 1),
        be1=np.asarray(inputs["be1"], np.float32).reshape(-1, 1),
    )
    in_maps = [dict(per_core[c], **shared) for c in range(cfg.ncores)]
    res = run_bass_kernel_spmd(nc, in_maps, core_ids=list(range(cfg.ncores)))
    out_pad = np.concatenate([res.results[c]["out"] for c in range(cfg.ncores)],
                             axis=0)
    return out_pad[newpos].astype(np.float32)


def kernel(x, edge_index, edge_weight, W0, b0, g0, be0, W1, b1, g1, be1,
           W2, b2):
    cfg = Cfg()
    return _run(cfg, dict(x=x, edge_index=edge_index, edge_weight=edge_weight,
                          W0=W0, W1=W1, W2=W2, b2=b2, g0=g0, be0=be0, g1=g1,
                          be1=be1))

